# revision 19
# baseline (speedup 1.0000x reference)
"""Trainium2 Bass kernel for nn_CustomConv: 3x3 same-padding conv.

Full problem: input [32, 32, 128, 128] f32, weight [64, 32, 3, 3] f32
-> output [32, 64, 128, 128] f32.

Sharding: data-parallel across 8 NeuronCores on the batch axis (4 images
per core); the small weight tensor is replicated.

Per-core design ("rho row-pair" scheme; both DMA bytes and PE columns at
their structural minimum):
  * Contraction K = 128 = (rho, ci): rho in 0..3 indexes a 4-row input
    window, ci the 32 input channels. M = 128 = (s, co): each streamed
    rhs column produces BOTH output rows of a row pair (s in {0,1}) for
    all 64 output channels -> full 128-wide PE array from a single
    stream. The 3 dx taps are 3 PSUM-accumulating passes whose rhs is
    the same buffer offset by dx elements. Weights are block-banded
    (dy = rho - s in {0,1,2}), so 3 passes x 32768 pair-columns
    = 98k PE cycles/core, the same column count as a perfectly paired
    M=64 scheme but without relying on column-group concurrency.
  * Storage: partition (rho, ci) holds 130-wide stored rows
    [0 | row | 0] of image rows 2k-1+rho (every other row); the zero
    columns make all 3 dx passes read valid data (no wrap), and the
    zero pad rows live in DRAM. rho in {0,1} (65 rows/image) is loaded
    from HBM ONCE; rho in {2,3} is the SAME data shifted one stored
    row, generated on-chip by a single flat 64-partition row-aligned
    vector copy per unit (4x DVE perf-mode eligible; measured cheap).
    No memsets, no misaligned copies, no gpsimd (its big copies
    measured 10x slow and its activity stalls concurrent DVE).
  * HBM traffic: 4.3 MiB loads + 8.4 MiB f16 stores per core (vs
    21.5 MB for the 3-copy baseline, which was DMA-bound at ~21 GB/s
    per SDMA engine).
  * PSUM: one 4-bank [128,2048] tile per 32-row store group (bufs=2),
    evicted as two [128,1024] casts split DVE/ACT (halves the
    per-instruction eviction overhead vs four [128,512]).
  * ~4us of dependency-free junk matmuls at kernel start warm the PE
    HAM clock gate during the pipeline-fill loads.
"""

import numpy as np

import concourse.bass as bass
import concourse.mybir as mybir
from concourse.tile import TileContext

F32 = mybir.dt.float32
F16 = mybir.dt.float16

B, CIN, H, W = 32, 32, 128, 128
COUT, KS = 64, 3
NCORES = 8
BPC = B // NCORES  # images per core
WR = W + 2  # stored-row width: [0 | row | 0]
NK = H // 2 + 1  # stored rows per rho in {0,1} per image

_CACHE = {}


def build_nc(bpc=BPC, h=H, split_waits=True):
    """Build the per-core Bass module. bpc/h are parameterized only for
    small-scale simulation tests; hardware uses the defaults."""
    assert h % 64 == 0
    npair = h // 2
    n_st = h // 32  # store groups of 32 output rows
    nk = npair + 1  # stored rows per rho01 partition per image
    nc = bass.Bass()
    x = nc.declare_dram_parameter("x", [bpc, 2, CIN, nk * WR], F16, isOutput=False)
    wts = nc.declare_dram_parameter("w", [128, 384], F16, isOutput=False)
    # Staged output layout (host un-permutes):
    # y[b, g, 64*s + co, 512*q + 128*tq + x] = out[b, co, 32g+8q+2tq+s, x]
    y = nc.declare_dram_parameter("y", [bpc, n_st, 128, 2048], F16, isOutput=True)

    x_ap = x.ap().rearrange("b r c s -> b (r c) s")  # [bpc, 64, nk*WR]
    y_ap = y.ap()

    with TileContext(nc) as tc:
        with (
            tc.tile_pool(name="wpool", bufs=1) as wpool,
            tc.tile_pool(name="scpool", bufs=1) as scpool,
            tc.tile_pool(name="inpool", bufs=6) as inpool,
            tc.tile_pool(name="stpool", bufs=6) as stpool,
            tc.tile_pool(name="psum", bufs=2, space="PSUM") as psum_pool,
        ):
            wt = wpool.tile([128, 384], F16)
            nc.sync.dma_start(out=wt, in_=wts.ap())

            # HAM warmup: junk matmuls on an UNINITIALIZED scratch tile
            # (zero data deps, results discarded) run the moment the
            # engines come live, so the PE clock gate is already 8/8 when
            # the first real matmul's data lands (~3us later).
            scr = scpool.tile([128, 384], F16)
            nc.gpsimd.memset(scr[:, :], 0.0)
            wps = psum_pool.tile([128, 2048], F32, tag="ps", name="warm")
            for i in range(14):
                nc.tensor.matmul(
                    wps[:, 0:384],
                    lhsT=scr[:, 0:128],
                    rhs=scr,
                    start=True,
                    stop=True,
                    skip_group_check=True,
                )

            # Pipeline units (in units of output row-pairs): half-images,
            # with the first and last half-image split into quarters so
            # the first matmul starts after ~a quarter of the fill load
            # and the drain tail stores sooner.
            hp = npair // 2
            def units_for(b):
                if hp < 16:
                    return [(0, hp), (hp, hp)]
                first = [(0, 16), (16, 16)] if b == 0 else [(0, hp)]
                second = (
                    [(hp, 16), (hp + 16, 16)] if b == bpc - 1 else [(hp, hp)]
                )
                return first + second

            for b in range(bpc):
                for t0, np_ in units_for(b):
                    lsz = (np_ + 1) * WR  # elems loaded per rho01 partition
                    hnp = np_ // 2
                    buf = inpool.tile([128, lsz], F16, tag="img")
                    # Row-aligned half-unit load chunks + half-copies:
                    # store group gl only depends on the halves it reads,
                    # so its matmuls start one chunk earlier.
                    cuts = [0, (hnp + 1) * WR, lsz]
                    for k in range(2):
                        nc.sync.dma_start(
                            out=buf[0:64, cuts[k] : cuts[k + 1]],
                            in_=x_ap[b][
                                :, t0 * WR + cuts[k] : t0 * WR + cuts[k + 1]
                            ],
                        )
                    # rho2/rho3 = rho0/rho1 advanced one stored row: flat
                    # 64-partition row-aligned copies (260B offset, 4x DVE
                    # perf-mode eligible).
                    nc.vector.tensor_copy(
                        out=buf[64:128, 0 : hnp * WR],
                        in_=buf[0:64, WR : (hnp + 1) * WR],
                    )
                    nc.vector.tensor_copy(
                        out=buf[64:128, hnp * WR : np_ * WR],
                        in_=buf[0:64, (hnp + 1) * WR : (np_ + 1) * WR],
                    )
                    bufv = buf.rearrange("p (k j) -> p k j", j=WR)

                    for gl in range(np_ // 16):
                        g = t0 // 16 + gl
                        st = stpool.tile([128, 2048], F16, tag="st")
                        ps = psum_pool.tile([128, 2048], F32, tag="ps")
                        # q-outer / dx-inner: each PSUM bank finishes in 3
                        # consecutive matmuls, so the two eviction halves
                        # and the two half-stores pipeline under the
                        # group's own matmuls (shorter drain tail, psum
                        # tile freed sooner).
                        for q in range(4):
                            for dx in range(3):
                                kloc = 16 * gl + 4 * q
                                nc.tensor.matmul(
                                    ps[:, 512 * q : 512 * q + 512],
                                    lhsT=wt[:, 128 * dx : 128 * dx + 128],
                                    rhs=bufv[:, kloc : kloc + 4, dx : dx + W],
                                    start=(dx == 0),
                                    stop=(dx == 2),
                                    skip_group_check=True,
                                )
                            if q == 1:
                                nc.vector.tensor_copy(
                                    out=st[:, 0:1024], in_=ps[:, 0:1024]
                                )
                                # Issue from the scalar queue: a sync-queue
                                # store's sem-wait would block the loads
                                # FIFO'd behind it (measured 1.2-1.6us
                                # stall per group).
                                nc.scalar.dma_start(
                                    out=y_ap[b, g][:, 0:1024],
                                    in_=st[:, 0:1024],
                                )
                            elif q == 3:
                                nc.scalar.copy(
                                    st[:, 1024:2048], ps[:, 1024:2048]
                                )
                                nc.scalar.dma_start(
                                    out=y_ap[b, g][:, 1024:2048],
                                    in_=st[:, 1024:2048],
                                )
    if split_waits:
        _split_waits(nc)
    return nc


# Per-instruction-struct HW sync-wait slot limits are small (walrus
# "Too many sync wait commands"). Split excess waits onto standalone
# NoOp instructions queued just before, on the same engine.
_WAIT_LIMIT = {}
_SKIP_SPLIT = {
    "InstEventSemaphore",
    "InstAllEngineBarrier",
    "InstUnconditionalBranch",
    "InstNoOp",
}


def _split_waits(nc):
    n = 0
    for f in nc.m.functions:
        for blk in f.blocks:
            new = []
            for inst in blk.instructions:
                si = getattr(inst, "sync_info", None)
                tname = type(inst).__name__
                if si is not None and si.on_wait and tname not in _SKIP_SPLIT:
                    limit = _WAIT_LIMIT.get(tname, 1)
                    if len(si.on_wait) > limit:
                        extra, keep = si.on_wait[:-limit], si.on_wait[-limit:]
                        for w in extra:
                            n += 1
                            new.append(
                                mybir.InstNoOp(
                                    name=f"wsplit-{n}",
                                    engine=inst.engine,
                                    sync_info=mybir.SyncInfo(
                                        on_wait=[w], on_update=[]
                                    ),
                                    bass_nofuse=True,
                                )
                            )
                        inst.sync_info = mybir.SyncInfo(
                            on_wait=keep, on_update=si.on_update
                        )
                new.append(inst)
            blk.instructions[:] = new
    return n


def _prep_weights(kernel):
    # wt[32*rho + ci, 128*dx + 64*s + co] = kernel[co, ci, rho - s, dx]
    # for rho - s in {0,1,2}, else 0.
    w = np.asarray(kernel).astype(np.float16)  # [co, ci, dy, dx]
    wt = np.zeros((128, 384), dtype=np.float16)
    for rho in range(4):
        for s in range(2):
            dy = rho - s
            if 0 <= dy <= 2:
                for dx in range(3):
                    # [co, ci] -> [ci, co]
                    wt[
                        32 * rho : 32 * rho + 32,
                        128 * dx + 64 * s : 128 * dx + 64 * s + 64,
                    ] = w[:, :, dy, dx].T
    return wt


def _prep_input(input, bpc=BPC, h=H):
    # [N, CIN, h, W] f32 -> f16 [N//bpc, bpc, 2, CIN, (h//2+1)*WR]:
    # padded[ci, 1+r, 1+x] = in[ci, r, x] (zero border, rows -1..h,
    # cols -1..W); x[n, rho, ci, k*WR : (k+1)*WR] = padded[ci, 2k+rho, :]
    # i.e. image row 2k-1+rho as a 130-wide stored row.
    n = input.shape[0]
    nk = h // 2 + 1
    pad = np.zeros((n, CIN, h + 2, WR), dtype=np.float16)
    pad[:, :, 1 : h + 1, 1 : W + 1] = input
    xr = np.empty((n, 2, CIN, nk, WR), dtype=np.float16)
    for rho in range(2):
        xr[:, rho] = pad[:, :, rho : rho + 2 * nk : 2, :].transpose(0, 1, 2, 3)
    return np.ascontiguousarray(xr.reshape(n // bpc, bpc, 2, CIN, nk * WR))


def run(input, kernel, **spmd_kwargs):
    """Run the kernel on 8 NeuronCores; returns (output, BassKernelResults)."""
    from concourse.bass_utils import run_bass_kernel_spmd

    if "nc" not in _CACHE:
        _CACHE["nc"] = build_nc()
    nc = _CACHE["nc"]

    inp = _prep_input(np.asarray(input))
    wts = _prep_weights(np.asarray(kernel))
    in_maps = [{"x": inp[c], "w": wts} for c in range(NCORES)]
    bkr = run_bass_kernel_spmd(nc, in_maps, list(range(NCORES)), **spmd_kwargs)
    out = np.concatenate([bkr.results[c]["y"] for c in range(NCORES)], axis=0)
    return _unstage(out), bkr


def _unstage(y, h=H):
    # y [B, n_st, 128, 2048] f16 -> out [B, COUT, h, W] f32
    # y[b, g, 64s+co, 512q + 128tq + x] = out[b, co, 32g+8q+2tq+s, x]
    n = y.shape[0]
    a = y.reshape(n, h // 32, 2, 64, 4, 4, W)  # b, g, s, co, q, tq, x
    a = a.transpose(0, 3, 1, 4, 5, 2, 6)  # b, co, g, q, tq, s, x
    return np.ascontiguousarray(a.reshape(n, COUT, h, W), dtype=np.float32)


def kernel(input, kernel):
    return run(input, kernel)[0]


# revision 20
# speedup vs baseline: 1.1515x; 1.1515x over previous
"""Trainium2 Bass kernel for nn_CustomConv: 3x3 same-padding conv.

Full problem: input [32, 32, 128, 128] f32, weight [64, 32, 3, 3] f32
-> output [32, 64, 128, 128] f32.

Sharding: data-parallel across 8 NeuronCores on the batch axis (4 images
per core); the small weight tensor is replicated.

Per-core design ("rho row-pair" scheme; both DMA bytes and PE columns at
their structural minimum):
  * Contraction K = 128 = (rho, ci): rho in 0..3 indexes a 4-row input
    window, ci the 32 input channels. M = 128 = (s, co): each streamed
    rhs column produces BOTH output rows of a row pair (s in {0,1}) for
    all 64 output channels -> full 128-wide PE array from a single
    stream. The 3 dx taps are 3 PSUM-accumulating passes whose rhs is
    the same buffer offset by dx elements. Weights are block-banded
    (dy = rho - s in {0,1,2}), so 3 passes x 32768 pair-columns
    = 98k PE cycles/core, the same column count as a perfectly paired
    M=64 scheme but without relying on column-group concurrency.
  * Storage: partition (rho, ci) holds 130-wide stored rows
    [0 | row | 0] of image rows 2k-1+rho (every other row); the zero
    columns make all 3 dx passes read valid data (no wrap), and the
    zero pad rows live in DRAM. rho in {0,1} (65 rows/image) is loaded
    from HBM ONCE; rho in {2,3} is the SAME data shifted one stored
    row, generated on-chip by a single flat 64-partition row-aligned
    vector copy per unit (4x DVE perf-mode eligible; measured cheap).
    No memsets, no misaligned copies, no gpsimd (its big copies
    measured 10x slow and its activity stalls concurrent DVE).
  * HBM traffic: 4.3 MiB loads + 8.4 MiB f16 stores per core (vs
    21.5 MB for the 3-copy baseline, which was DMA-bound at ~21 GB/s
    per SDMA engine).
  * PSUM: one 4-bank [128,2048] tile per 32-row store group (bufs=2),
    evicted as two [128,1024] casts split DVE/ACT (halves the
    per-instruction eviction overhead vs four [128,512]).
  * ~4us of dependency-free junk matmuls at kernel start warm the PE
    HAM clock gate during the pipeline-fill loads.
"""

import numpy as np

import concourse.bass as bass
import concourse.mybir as mybir
from concourse.tile import TileContext

F32 = mybir.dt.float32
F16 = mybir.dt.float16

B, CIN, H, W = 32, 32, 128, 128
COUT, KS = 64, 3
NCORES = 8
BPC = B // NCORES  # images per core
WR = W + 2  # stored-row width: [0 | row | 0]
NK = H // 2 + 1  # stored rows per rho in {0,1} per image

_CACHE = {}


def build_nc(bpc=BPC, h=H, split_waits=True):
    """Build the per-core Bass module. bpc/h are parameterized only for
    small-scale simulation tests; hardware uses the defaults."""
    assert h % 64 == 0
    npair = h // 2
    n_st = h // 32  # store groups of 32 output rows
    nk = npair + 1  # stored rows per rho01 partition per image
    nc = bass.Bass()
    x = nc.declare_dram_parameter("x", [bpc, 2, CIN, nk * WR], F16, isOutput=False)
    wts = nc.declare_dram_parameter("w", [128, 384], F16, isOutput=False)
    # Staged output layout (host un-permutes):
    # y[b, g, 64*s + co, 512*q + 128*tq + x] = out[b, co, 32g+8q+2tq+s, x]
    y = nc.declare_dram_parameter("y", [bpc, n_st, 128, 2048], F16, isOutput=True)

    x_ap = x.ap().rearrange("b r c s -> b (r c) s")  # [bpc, 64, nk*WR]
    y_ap = y.ap()

    with TileContext(nc) as tc:
        with (
            tc.tile_pool(name="wpool", bufs=1) as wpool,
            tc.tile_pool(name="scpool", bufs=1) as scpool,
            tc.tile_pool(name="inpool", bufs=6) as inpool,
            tc.tile_pool(name="stpool", bufs=6) as stpool,
            tc.tile_pool(name="psum", bufs=2, space="PSUM") as psum_pool,
        ):
            wt = wpool.tile([128, 384], F16)
            nc.sync.dma_start(out=wt, in_=wts.ap())

            # HAM warmup: junk matmuls on an UNINITIALIZED scratch tile
            # (zero data deps, results discarded) run the moment the
            # engines come live, so the PE clock gate is already 8/8 when
            # the first real matmul's data lands (~3us later).
            scr = scpool.tile([128, 384], F16)
            nc.gpsimd.memset(scr[:, :], 0.0)
            wps = psum_pool.tile([128, 2048], F32, tag="ps", name="warm")
            for i in range(14):
                nc.tensor.matmul(
                    wps[:, 0:384],
                    lhsT=scr[:, 0:128],
                    rhs=scr,
                    start=True,
                    stop=True,
                    skip_group_check=True,
                )

            # Pipeline units (in units of output row-pairs): half-images,
            # with the first and last half-image split into quarters so
            # the first matmul starts after ~a quarter of the fill load
            # and the drain tail stores sooner.
            hp = npair // 2
            def units_for(b):
                if hp < 16:
                    return [(0, hp), (hp, hp)]
                first = [(0, 16), (16, 16)] if b == 0 else [(0, hp)]
                second = (
                    [(hp, 16), (hp + 16, 16)] if b == bpc - 1 else [(hp, hp)]
                )
                return first + second

            for b in range(bpc):
                for t0, np_ in units_for(b):
                    lsz = (np_ + 1) * WR  # elems loaded per rho01 partition
                    hnp = np_ // 2
                    buf = inpool.tile([128, lsz], F16, tag="img")
                    # Row-aligned half-unit load chunks + half-copies:
                    # store group gl only depends on the halves it reads,
                    # so its matmuls start one chunk earlier.
                    cuts = [0, (hnp + 1) * WR, lsz]
                    for k in range(2):
                        nc.sync.dma_start(
                            out=buf[0:64, cuts[k] : cuts[k + 1]],
                            in_=x_ap[b][
                                :, t0 * WR + cuts[k] : t0 * WR + cuts[k + 1]
                            ],
                        )
                    # rho2/rho3 = rho0/rho1 advanced one stored row: flat
                    # 64-partition row-aligned copies (260B offset, 4x DVE
                    # perf-mode eligible).
                    nc.vector.tensor_copy(
                        out=buf[64:128, 0 : hnp * WR],
                        in_=buf[0:64, WR : (hnp + 1) * WR],
                    )
                    nc.vector.tensor_copy(
                        out=buf[64:128, hnp * WR : np_ * WR],
                        in_=buf[0:64, (hnp + 1) * WR : (np_ + 1) * WR],
                    )
                    bufv = buf.rearrange("p (k j) -> p k j", j=WR)

                    for gl in range(np_ // 16):
                        g = t0 // 16 + gl
                        last = b == bpc - 1 and g == n_st - 1
                        st = stpool.tile([128, 2048], F16, tag="st")
                        ps = psum_pool.tile([128, 2048], F32, tag="ps")
                        # dx-outer keeps the static schedule clean (a
                        # q-outer early-evict variant measured WORSE: the
                        # list scheduler interleaves groups and delays the
                        # evictions the psum rotation waits on). Only the
                        # LAST group uses q-outer + split stores to
                        # shorten the drain tail (no competing groups
                        # left to mis-schedule).
                        order = (
                            [(q, dx) for q in range(4) for dx in range(3)]
                            if last
                            else [(q, dx) for dx in range(3) for q in range(4)]
                        )
                        for q, dx in order:
                            kloc = 16 * gl + 4 * q
                            nc.tensor.matmul(
                                ps[:, 512 * q : 512 * q + 512],
                                lhsT=wt[:, 128 * dx : 128 * dx + 128],
                                rhs=bufv[:, kloc : kloc + 4, dx : dx + W],
                                start=(dx == 0),
                                stop=(dx == 2),
                                skip_group_check=True,
                            )
                            if last and q == 1 and dx == 2:
                                nc.vector.tensor_copy(
                                    out=st[:, 0:1024], in_=ps[:, 0:1024]
                                )
                                nc.sync.dma_start(
                                    out=y_ap[b, g][:, 0:1024],
                                    in_=st[:, 0:1024],
                                )
                        if last:
                            nc.scalar.copy(st[:, 1024:2048], ps[:, 1024:2048])
                            nc.scalar.dma_start(
                                out=y_ap[b, g][:, 1024:2048],
                                in_=st[:, 1024:2048],
                            )
                        else:
                            nc.vector.tensor_copy(
                                out=st[:, 0:1024], in_=ps[:, 0:1024]
                            )
                            nc.scalar.copy(st[:, 1024:2048], ps[:, 1024:2048])
                            nc.scalar.dma_start(out=y_ap[b, g], in_=st)
    if split_waits:
        _split_waits(nc)
    return nc


# Per-instruction-struct HW sync-wait slot limits are small (walrus
# "Too many sync wait commands"). Split excess waits onto standalone
# NoOp instructions queued just before, on the same engine.
_WAIT_LIMIT = {}
_SKIP_SPLIT = {
    "InstEventSemaphore",
    "InstAllEngineBarrier",
    "InstUnconditionalBranch",
    "InstNoOp",
}


def _split_waits(nc):
    n = 0
    for f in nc.m.functions:
        for blk in f.blocks:
            new = []
            for inst in blk.instructions:
                si = getattr(inst, "sync_info", None)
                tname = type(inst).__name__
                if si is not None and si.on_wait and tname not in _SKIP_SPLIT:
                    limit = _WAIT_LIMIT.get(tname, 1)
                    if len(si.on_wait) > limit:
                        extra, keep = si.on_wait[:-limit], si.on_wait[-limit:]
                        for w in extra:
                            n += 1
                            new.append(
                                mybir.InstNoOp(
                                    name=f"wsplit-{n}",
                                    engine=inst.engine,
                                    sync_info=mybir.SyncInfo(
                                        on_wait=[w], on_update=[]
                                    ),
                                    bass_nofuse=True,
                                )
                            )
                        inst.sync_info = mybir.SyncInfo(
                            on_wait=keep, on_update=si.on_update
                        )
                new.append(inst)
            blk.instructions[:] = new
    return n


def _prep_weights(kernel):
    # wt[32*rho + ci, 128*dx + 64*s + co] = kernel[co, ci, rho - s, dx]
    # for rho - s in {0,1,2}, else 0.
    w = np.asarray(kernel).astype(np.float16)  # [co, ci, dy, dx]
    wt = np.zeros((128, 384), dtype=np.float16)
    for rho in range(4):
        for s in range(2):
            dy = rho - s
            if 0 <= dy <= 2:
                for dx in range(3):
                    # [co, ci] -> [ci, co]
                    wt[
                        32 * rho : 32 * rho + 32,
                        128 * dx + 64 * s : 128 * dx + 64 * s + 64,
                    ] = w[:, :, dy, dx].T
    return wt


def _prep_input(input, bpc=BPC, h=H):
    # [N, CIN, h, W] f32 -> f16 [N//bpc, bpc, 2, CIN, (h//2+1)*WR]:
    # padded[ci, 1+r, 1+x] = in[ci, r, x] (zero border, rows -1..h,
    # cols -1..W); x[n, rho, ci, k*WR : (k+1)*WR] = padded[ci, 2k+rho, :]
    # i.e. image row 2k-1+rho as a 130-wide stored row.
    n = input.shape[0]
    nk = h // 2 + 1
    pad = np.zeros((n, CIN, h + 2, WR), dtype=np.float16)
    pad[:, :, 1 : h + 1, 1 : W + 1] = input
    xr = np.empty((n, 2, CIN, nk, WR), dtype=np.float16)
    for rho in range(2):
        xr[:, rho] = pad[:, :, rho : rho + 2 * nk : 2, :].transpose(0, 1, 2, 3)
    return np.ascontiguousarray(xr.reshape(n // bpc, bpc, 2, CIN, nk * WR))


def run(input, kernel, **spmd_kwargs):
    """Run the kernel on 8 NeuronCores; returns (output, BassKernelResults)."""
    from concourse.bass_utils import run_bass_kernel_spmd

    if "nc" not in _CACHE:
        _CACHE["nc"] = build_nc()
    nc = _CACHE["nc"]

    inp = _prep_input(np.asarray(input))
    wts = _prep_weights(np.asarray(kernel))
    in_maps = [{"x": inp[c], "w": wts} for c in range(NCORES)]
    bkr = run_bass_kernel_spmd(nc, in_maps, list(range(NCORES)), **spmd_kwargs)
    out = np.concatenate([bkr.results[c]["y"] for c in range(NCORES)], axis=0)
    return _unstage(out), bkr


def _unstage(y, h=H):
    # y [B, n_st, 128, 2048] f16 -> out [B, COUT, h, W] f32
    # y[b, g, 64s+co, 512q + 128tq + x] = out[b, co, 32g+8q+2tq+s, x]
    n = y.shape[0]
    a = y.reshape(n, h // 32, 2, 64, 4, 4, W)  # b, g, s, co, q, tq, x
    a = a.transpose(0, 3, 1, 4, 5, 2, 6)  # b, co, g, q, tq, s, x
    return np.ascontiguousarray(a.reshape(n, COUT, h, W), dtype=np.float32)


def kernel(input, kernel):
    return run(input, kernel)[0]


# revision 22
# speedup vs baseline: 1.1820x; 1.0265x over previous
"""Trainium2 Bass kernel for nn_CustomConv: 3x3 same-padding conv.

Full problem: input [32, 32, 128, 128] f32, weight [64, 32, 3, 3] f32
-> output [32, 64, 128, 128] f32.

Sharding: data-parallel across 8 NeuronCores on the batch axis (4 images
per core); the small weight tensor is replicated.

Per-core design ("rho row-pair" scheme; both DMA bytes and PE columns at
their structural minimum):
  * Contraction K = 128 = (rho, ci): rho in 0..3 indexes a 4-row input
    window, ci the 32 input channels. M = 128 = (s, co): each streamed
    rhs column produces BOTH output rows of a row pair (s in {0,1}) for
    all 64 output channels -> full 128-wide PE array from a single
    stream. The 3 dx taps are 3 PSUM-accumulating passes whose rhs is
    the same buffer offset by dx elements. Weights are block-banded
    (dy = rho - s in {0,1,2}), so 3 passes x 32768 pair-columns
    = 98k PE cycles/core, the same column count as a perfectly paired
    M=64 scheme but without relying on column-group concurrency.
  * Storage: partition (rho, ci) holds 130-wide stored rows
    [0 | row | 0] of image rows 2k-1+rho (every other row); the zero
    columns make all 3 dx passes read valid data (no wrap), and the
    zero pad rows live in DRAM. rho in {0,1} (65 rows/image) is loaded
    from HBM ONCE; rho in {2,3} is the SAME data shifted one stored
    row, generated on-chip by a single flat 64-partition row-aligned
    vector copy per unit (4x DVE perf-mode eligible; measured cheap).
    No memsets, no misaligned copies, no gpsimd (its big copies
    measured 10x slow and its activity stalls concurrent DVE).
  * HBM traffic: 4.3 MiB loads + 8.4 MiB f16 stores per core (vs
    21.5 MB for the 3-copy baseline, which was DMA-bound at ~21 GB/s
    per SDMA engine).
  * PSUM: one 4-bank [128,2048] tile per 32-row store group (bufs=2),
    evicted as two [128,1024] casts split DVE/ACT (halves the
    per-instruction eviction overhead vs four [128,512]).
  * ~4us of dependency-free junk matmuls at kernel start warm the PE
    HAM clock gate during the pipeline-fill loads.
"""

import numpy as np

import concourse.bass as bass
import concourse.mybir as mybir
from concourse.tile import TileContext

F32 = mybir.dt.float32
F16 = mybir.dt.float16

B, CIN, H, W = 32, 32, 128, 128
COUT, KS = 64, 3
NCORES = 8
BPC = B // NCORES  # images per core
WR = W + 2  # stored-row width: [0 | row | 0]
NK = H // 2 + 1  # stored rows per rho in {0,1} per image

_CACHE = {}


def build_nc(bpc=BPC, h=H, split_waits=True):
    """Build the per-core Bass module. bpc/h are parameterized only for
    small-scale simulation tests; hardware uses the defaults."""
    assert h % 64 == 0
    npair = h // 2
    n_st = h // 32  # store groups of 32 output rows
    nk = npair + 1  # stored rows per rho01 partition per image
    nc = bass.Bass()
    x = nc.declare_dram_parameter("x", [bpc, 2, CIN, nk * WR], F16, isOutput=False)
    wts = nc.declare_dram_parameter("w", [128, 384], F16, isOutput=False)
    # Staged output layout (host un-permutes):
    # y[b, g, 64*s + co, 512*q + 128*tq + x] = out[b, co, 32g+8q+2tq+s, x]
    y = nc.declare_dram_parameter("y", [bpc, n_st, 128, 2048], F16, isOutput=True)

    x_ap = x.ap().rearrange("b r c s -> b (r c) s")  # [bpc, 64, nk*WR]
    y_ap = y.ap()

    with TileContext(nc) as tc:
        with (
            tc.tile_pool(name="wpool", bufs=1) as wpool,
            tc.tile_pool(name="scpool", bufs=1) as scpool,
            tc.tile_pool(name="inpool", bufs=6) as inpool,
            tc.tile_pool(name="stpool", bufs=6) as stpool,
            tc.tile_pool(name="psum", bufs=2, space="PSUM") as psum_pool,
        ):
            wt = wpool.tile([128, 384], F16)
            nc.sync.dma_start(out=wt, in_=wts.ap())

            # HAM warmup: junk matmuls on an UNINITIALIZED scratch tile
            # (zero data deps, results discarded) run the moment the
            # engines come live, so the PE clock gate is already 8/8 when
            # the first real matmul's data lands (~3us later).
            scr = scpool.tile([128, 384], F16)
            nc.gpsimd.memset(scr[:, :], 0.0)
            wps = psum_pool.tile([128, 2048], F32, tag="ps", name="warm")
            for i in range(14):
                nc.tensor.matmul(
                    wps[:, 0:384],
                    lhsT=scr[:, 0:128],
                    rhs=scr,
                    start=True,
                    stop=True,
                    skip_group_check=True,
                )

            # Pipeline units (in units of output row-pairs): half-images,
            # with the first and last half-image split into quarters so
            # the first matmul starts after ~a quarter of the fill load
            # and the drain tail stores sooner.
            hp = npair // 2
            def units_for(b):
                if hp < 16:
                    return [(0, hp), (hp, hp)]
                first = [(0, 16), (16, 16)] if b == 0 else [(0, hp)]
                second = (
                    [(hp, 16), (hp + 16, 16)] if b == bpc - 1 else [(hp, hp)]
                )
                return first + second

            units = [
                (b, t0, np_) for b in range(bpc) for t0, np_ in units_for(b)
            ]

            def fetch(u):
                """Issue unit u's loads and rho23 shift-copies; return the
                buffer tile. Called one unit AHEAD of the matmul groups so
                the DVE copies aren't FIFO'd behind the previous unit's
                psum casts (measured 0.5-2.4us/unit of PE stall)."""
                b, t0, np_ = u
                lsz = (np_ + 1) * WR
                hnp = np_ // 2
                buf = inpool.tile([128, lsz], F16, tag="img", name="buf")
                # Row-aligned half-unit load chunks + half-copies: store
                # group gl only depends on the halves it reads.
                cuts = [0, (hnp + 1) * WR, lsz]
                for k in range(2):
                    nc.sync.dma_start(
                        out=buf[0:64, cuts[k] : cuts[k + 1]],
                        in_=x_ap[b][
                            :, t0 * WR + cuts[k] : t0 * WR + cuts[k + 1]
                        ],
                    )
                # rho2/rho3 = rho0/rho1 advanced one stored row: flat
                # 64-partition row-aligned copies (260B offset, 4x DVE
                # perf-mode eligible).
                nc.vector.tensor_copy(
                    out=buf[64:128, 0 : hnp * WR],
                    in_=buf[0:64, WR : (hnp + 1) * WR],
                )
                nc.vector.tensor_copy(
                    out=buf[64:128, hnp * WR : np_ * WR],
                    in_=buf[0:64, (hnp + 1) * WR : (np_ + 1) * WR],
                )
                return buf

            nbuf = fetch(units[0])
            for ui, (b, t0, np_) in enumerate(units):
                buf = nbuf
                if ui + 1 < len(units):
                    nbuf = fetch(units[ui + 1])
                bufv = buf.rearrange("p (k j) -> p k j", j=WR)
                if True:
                    for gl in range(np_ // 16):
                        g = t0 // 16 + gl
                        last = b == bpc - 1 and g == n_st - 1
                        st = stpool.tile([128, 2048], F16, tag="st")
                        ps = psum_pool.tile([128, 2048], F32, tag="ps")
                        # dx-outer keeps the static schedule clean (a
                        # q-outer early-evict variant measured WORSE: the
                        # list scheduler interleaves groups and delays the
                        # evictions the psum rotation waits on). Only the
                        # LAST group uses q-outer + split stores to
                        # shorten the drain tail (no competing groups
                        # left to mis-schedule).
                        order = (
                            [(q, dx) for q in range(4) for dx in range(3)]
                            if last
                            else [(q, dx) for dx in range(3) for q in range(4)]
                        )
                        for q, dx in order:
                            kloc = 16 * gl + 4 * q
                            nc.tensor.matmul(
                                ps[:, 512 * q : 512 * q + 512],
                                lhsT=wt[:, 128 * dx : 128 * dx + 128],
                                rhs=bufv[:, kloc : kloc + 4, dx : dx + W],
                                start=(dx == 0),
                                stop=(dx == 2),
                                skip_group_check=True,
                            )
                            if last and q == 1 and dx == 2:
                                nc.vector.tensor_copy(
                                    out=st[:, 0:1024], in_=ps[:, 0:1024]
                                )
                                nc.sync.dma_start(
                                    out=y_ap[b, g][:, 0:1024],
                                    in_=st[:, 0:1024],
                                )
                        if last:
                            nc.scalar.copy(st[:, 1024:2048], ps[:, 1024:2048])
                            nc.scalar.dma_start(
                                out=y_ap[b, g][:, 1024:2048],
                                in_=st[:, 1024:2048],
                            )
                        else:
                            nc.vector.tensor_copy(
                                out=st[:, 0:1024], in_=ps[:, 0:1024]
                            )
                            nc.scalar.copy(st[:, 1024:2048], ps[:, 1024:2048])
                            nc.scalar.dma_start(out=y_ap[b, g], in_=st)
    if split_waits:
        _split_waits(nc)
    return nc


# Per-instruction-struct HW sync-wait slot limits are small (walrus
# "Too many sync wait commands"). Split excess waits onto standalone
# NoOp instructions queued just before, on the same engine.
_WAIT_LIMIT = {}
_SKIP_SPLIT = {
    "InstEventSemaphore",
    "InstAllEngineBarrier",
    "InstUnconditionalBranch",
    "InstNoOp",
}


def _split_waits(nc):
    n = 0
    for f in nc.m.functions:
        for blk in f.blocks:
            new = []
            for inst in blk.instructions:
                si = getattr(inst, "sync_info", None)
                tname = type(inst).__name__
                if si is not None and si.on_wait and tname not in _SKIP_SPLIT:
                    limit = _WAIT_LIMIT.get(tname, 1)
                    if len(si.on_wait) > limit:
                        extra, keep = si.on_wait[:-limit], si.on_wait[-limit:]
                        for w in extra:
                            n += 1
                            new.append(
                                mybir.InstNoOp(
                                    name=f"wsplit-{n}",
                                    engine=inst.engine,
                                    sync_info=mybir.SyncInfo(
                                        on_wait=[w], on_update=[]
                                    ),
                                    bass_nofuse=True,
                                )
                            )
                        inst.sync_info = mybir.SyncInfo(
                            on_wait=keep, on_update=si.on_update
                        )
                new.append(inst)
            blk.instructions[:] = new
    return n


def _prep_weights(kernel):
    # wt[32*rho + ci, 128*dx + 64*s + co] = kernel[co, ci, rho - s, dx]
    # for rho - s in {0,1,2}, else 0.
    w = np.asarray(kernel).astype(np.float16)  # [co, ci, dy, dx]
    wt = np.zeros((128, 384), dtype=np.float16)
    for rho in range(4):
        for s in range(2):
            dy = rho - s
            if 0 <= dy <= 2:
                for dx in range(3):
                    # [co, ci] -> [ci, co]
                    wt[
                        32 * rho : 32 * rho + 32,
                        128 * dx + 64 * s : 128 * dx + 64 * s + 64,
                    ] = w[:, :, dy, dx].T
    return wt


def _prep_input(input, bpc=BPC, h=H):
    # [N, CIN, h, W] f32 -> f16 [N//bpc, bpc, 2, CIN, (h//2+1)*WR]:
    # padded[ci, 1+r, 1+x] = in[ci, r, x] (zero border, rows -1..h,
    # cols -1..W); x[n, rho, ci, k*WR : (k+1)*WR] = padded[ci, 2k+rho, :]
    # i.e. image row 2k-1+rho as a 130-wide stored row.
    n = input.shape[0]
    nk = h // 2 + 1
    pad = np.zeros((n, CIN, h + 2, WR), dtype=np.float16)
    pad[:, :, 1 : h + 1, 1 : W + 1] = input
    xr = np.empty((n, 2, CIN, nk, WR), dtype=np.float16)
    for rho in range(2):
        xr[:, rho] = pad[:, :, rho : rho + 2 * nk : 2, :].transpose(0, 1, 2, 3)
    return np.ascontiguousarray(xr.reshape(n // bpc, bpc, 2, CIN, nk * WR))


def run(input, kernel, **spmd_kwargs):
    """Run the kernel on 8 NeuronCores; returns (output, BassKernelResults)."""
    from concourse.bass_utils import run_bass_kernel_spmd

    if "nc" not in _CACHE:
        _CACHE["nc"] = build_nc()
    nc = _CACHE["nc"]

    inp = _prep_input(np.asarray(input))
    wts = _prep_weights(np.asarray(kernel))
    in_maps = [{"x": inp[c], "w": wts} for c in range(NCORES)]
    bkr = run_bass_kernel_spmd(nc, in_maps, list(range(NCORES)), **spmd_kwargs)
    out = np.concatenate([bkr.results[c]["y"] for c in range(NCORES)], axis=0)
    return _unstage(out), bkr


def _unstage(y, h=H):
    # y [B, n_st, 128, 2048] f16 -> out [B, COUT, h, W] f32
    # y[b, g, 64s+co, 512q + 128tq + x] = out[b, co, 32g+8q+2tq+s, x]
    n = y.shape[0]
    a = y.reshape(n, h // 32, 2, 64, 4, 4, W)  # b, g, s, co, q, tq, x
    a = a.transpose(0, 3, 1, 4, 5, 2, 6)  # b, co, g, q, tq, s, x
    return np.ascontiguousarray(a.reshape(n, COUT, h, W), dtype=np.float32)


def kernel(input, kernel):
    return run(input, kernel)[0]


# revision 26
# speedup vs baseline: 1.2380x; 1.0474x over previous
"""Trainium2 Bass kernel for nn_CustomConv: 3x3 same-padding conv.

Full problem: input [32, 32, 128, 128] f32, weight [64, 32, 3, 3] f32
-> output [32, 64, 128, 128] f32.

Sharding: data-parallel across 8 NeuronCores on the batch axis (4 images
per core); the small weight tensor is replicated.

Per-core design ("rho row-pair" scheme; both DMA bytes and PE columns at
their structural minimum):
  * Contraction K = 128 = (rho, ci): rho in 0..3 indexes a 4-row input
    window, ci the 32 input channels. M = 128 = (s, co): each streamed
    rhs column produces BOTH output rows of a row pair (s in {0,1}) for
    all 64 output channels -> full 128-wide PE array from a single
    stream. The 3 dx taps are 3 PSUM-accumulating passes whose rhs is
    the same buffer offset by dx elements. Weights are block-banded
    (dy = rho - s in {0,1,2}), so 3 passes x 32768 pair-columns
    = 98k PE cycles/core, the same column count as a perfectly paired
    M=64 scheme but without relying on column-group concurrency.
  * Storage: partition (rho, ci) holds 130-wide stored rows
    [0 | row | 0] of image rows 2k-1+rho (every other row); the zero
    columns make all 3 dx passes read valid data (no wrap), and the
    zero pad rows live in DRAM. rho in {0,1} (65 rows/image) is loaded
    from HBM ONCE; rho in {2,3} is the SAME data shifted one stored
    row, generated on-chip by a single flat 64-partition row-aligned
    vector copy per unit (4x DVE perf-mode eligible; measured cheap).
    No memsets, no misaligned copies, no gpsimd (its big copies
    measured 10x slow and its activity stalls concurrent DVE).
  * HBM traffic: 4.3 MiB loads + 8.4 MiB f16 stores per core (vs
    21.5 MB for the 3-copy baseline, which was DMA-bound at ~21 GB/s
    per SDMA engine).
  * PSUM: one 4-bank [128,2048] tile per 32-row store group (bufs=2),
    evicted as two [128,1024] casts split DVE/ACT (halves the
    per-instruction eviction overhead vs four [128,512]).
  * ~4us of dependency-free junk matmuls at kernel start warm the PE
    HAM clock gate during the pipeline-fill loads.
"""

import numpy as np

import concourse.bass as bass
import concourse.mybir as mybir
from concourse.tile import TileContext

F32 = mybir.dt.float32
F16 = mybir.dt.float16

B, CIN, H, W = 32, 32, 128, 128
COUT, KS = 64, 3
NCORES = 8
BPC = B // NCORES  # images per core
WR = W + 2  # stored-row width: [0 | row | 0]
NK = H // 2 + 1  # stored rows per rho in {0,1} per image

_CACHE = {}


def build_nc(bpc=BPC, h=H, split_waits=True):
    """Build the per-core Bass module. bpc/h are parameterized only for
    small-scale simulation tests; hardware uses the defaults."""
    assert h % 64 == 0
    npair = h // 2
    n_st = h // 32  # store groups of 32 output rows
    nk = npair + 1  # stored rows per rho01 partition per image
    nc = bass.Bass()
    x = nc.declare_dram_parameter("x", [bpc, 2, CIN, nk * WR], F16, isOutput=False)
    wts = nc.declare_dram_parameter("w", [128, 384], F16, isOutput=False)
    # Staged output layout (host un-permutes):
    # y[b, g, 64*s + co, 512*q + 128*tq + x] = out[b, co, 32g+8q+2tq+s, x]
    y = nc.declare_dram_parameter("y", [bpc, n_st, 128, 2048], F16, isOutput=True)

    x_ap = x.ap().rearrange("b r c s -> b (r c) s")  # [bpc, 64, nk*WR]
    y_ap = y.ap()

    with TileContext(nc) as tc:
        with (
            tc.tile_pool(name="wpool", bufs=1) as wpool,
            tc.tile_pool(name="scpool", bufs=1) as scpool,
            tc.tile_pool(name="inpool", bufs=6) as inpool,
            tc.tile_pool(name="stpool", bufs=6) as stpool,
            tc.tile_pool(name="psum", bufs=4, space="PSUM") as psum_pool,
        ):
            wt = wpool.tile([128, 384], F16)
            nc.sync.dma_start(out=wt, in_=wts.ap())

            # HAM warmup: junk matmuls on an UNINITIALIZED scratch tile
            # (zero data deps, results discarded) run the moment the
            # engines come live, so the PE clock gate is already 8/8 when
            # the first real matmul's data lands (~3us later).
            scr = scpool.tile([128, 384], F16)
            nc.gpsimd.memset(scr[:, :], 0.0)
            wps = psum_pool.tile([128, 1024], F32, tag="ps", name="warm")
            for i in range(14):
                nc.tensor.matmul(
                    wps[:, 0:384],
                    lhsT=scr[:, 0:128],
                    rhs=scr,
                    start=True,
                    stop=True,
                    skip_group_check=True,
                )

            # Pipeline units (in units of output row-pairs): half-images,
            # with the first and last half-image split into quarters so
            # the first matmul starts after ~a quarter of the fill load
            # and the drain tail stores sooner.
            hp = npair // 2
            def units_for(b):
                if hp < 16:
                    return [(0, hp), (hp, hp)]
                first = [(0, 16), (16, 16)] if b == 0 else [(0, hp)]
                second = (
                    [(hp, 16), (hp + 16, 16)] if b == bpc - 1 else [(hp, hp)]
                )
                return first + second

            units = [
                (b, t0, np_) for b in range(bpc) for t0, np_ in units_for(b)
            ]

            def fetch(u):
                """Issue unit u's loads and rho23 shift-copies; return the
                buffer tile. Called one unit AHEAD of the matmul groups so
                the DVE copies aren't FIFO'd behind the previous unit's
                psum casts (measured 0.5-2.4us/unit of PE stall)."""
                b, t0, np_ = u
                lsz = (np_ + 1) * WR
                hnp = np_ // 2
                buf = inpool.tile([128, lsz], F16, tag="img", name="buf")
                # Row-aligned half-unit load chunks + half-copies: store
                # group gl only depends on the halves it reads.
                cuts = [0, (hnp + 1) * WR, lsz]
                for k in range(2):
                    nc.sync.dma_start(
                        out=buf[0:64, cuts[k] : cuts[k + 1]],
                        in_=x_ap[b][
                            :, t0 * WR + cuts[k] : t0 * WR + cuts[k + 1]
                        ],
                    )
                # rho2/rho3 = rho0/rho1 advanced one stored row: flat
                # 64-partition row-aligned copies (260B offset, 4x DVE
                # perf-mode eligible).
                nc.vector.tensor_copy(
                    out=buf[64:128, 0 : hnp * WR],
                    in_=buf[0:64, WR : (hnp + 1) * WR],
                )
                nc.vector.tensor_copy(
                    out=buf[64:128, hnp * WR : np_ * WR],
                    in_=buf[0:64, (hnp + 1) * WR : (np_ + 1) * WR],
                )
                return buf

            nbuf = fetch(units[0])
            for ui, (b, t0, np_) in enumerate(units):
                buf = nbuf
                if ui + 1 < len(units):
                    nbuf = fetch(units[ui + 1])
                bufv = buf.rearrange("p (k j) -> p k j", j=WR)
                if True:
                    for gl in range(np_ // 16):
                        g = t0 // 16 + gl
                        last = b == bpc - 1 and g == n_st - 1
                        st = stpool.tile([128, 2048], F16, tag="st")
                        # Two 2-bank psum tiles per group: q0/q1 banks are
                        # freed by the DVE cast ALONE and q2/q3 by the ACT
                        # activate ALONE, decoupling the WAR chain that a
                        # single 4-bank tile serializes on max(both).
                        psa = psum_pool.tile([128, 1024], F32, tag="ps", name="psa")
                        psb = psum_pool.tile([128, 1024], F32, tag="ps", name="psb")
                        # dx-outer keeps the static schedule clean (a
                        # q-outer early-evict variant measured WORSE: the
                        # list scheduler interleaves groups and delays the
                        # evictions the psum rotation waits on). Only the
                        # LAST group uses q-outer + split stores to
                        # shorten the drain tail (no competing groups
                        # left to mis-schedule).
                        order = (
                            [(q, dx) for q in range(4) for dx in range(3)]
                            if last
                            else [(q, dx) for dx in range(3) for q in range(4)]
                        )
                        for q, dx in order:
                            kloc = 16 * gl + 4 * q
                            psq = psa if q < 2 else psb
                            nc.tensor.matmul(
                                psq[:, 512 * (q % 2) : 512 * (q % 2) + 512],
                                lhsT=wt[:, 128 * dx : 128 * dx + 128],
                                rhs=bufv[:, kloc : kloc + 4, dx : dx + W],
                                start=(dx == 0),
                                stop=(dx == 2),
                                skip_group_check=True,
                            )
                            if last and q == 1 and dx == 2:
                                nc.vector.tensor_copy(
                                    out=st[:, 0:1024], in_=psa
                                )
                                nc.sync.dma_start(
                                    out=y_ap[b, g][:, 0:1024],
                                    in_=st[:, 0:1024],
                                )
                        if last:
                            nc.scalar.copy(st[:, 1024:2048], psb)
                            nc.scalar.dma_start(
                                out=y_ap[b, g][:, 1024:2048],
                                in_=st[:, 1024:2048],
                            )
                        else:
                            nc.vector.tensor_copy(out=st[:, 0:1024], in_=psa)
                            nc.scalar.copy(st[:, 1024:2048], psb)
                            nc.scalar.dma_start(out=y_ap[b, g], in_=st)
    if split_waits:
        _split_waits(nc)
    return nc


# Per-instruction-struct HW sync-wait slot limits are small (walrus
# "Too many sync wait commands"). Split excess waits onto standalone
# NoOp instructions queued just before, on the same engine.
_WAIT_LIMIT = {}
_SKIP_SPLIT = {
    "InstEventSemaphore",
    "InstAllEngineBarrier",
    "InstUnconditionalBranch",
    "InstNoOp",
}


def _split_waits(nc):
    n = 0
    for f in nc.m.functions:
        for blk in f.blocks:
            new = []
            for inst in blk.instructions:
                si = getattr(inst, "sync_info", None)
                tname = type(inst).__name__
                if si is not None and si.on_wait and tname not in _SKIP_SPLIT:
                    limit = _WAIT_LIMIT.get(tname, 1)
                    if len(si.on_wait) > limit:
                        extra, keep = si.on_wait[:-limit], si.on_wait[-limit:]
                        for w in extra:
                            n += 1
                            new.append(
                                mybir.InstNoOp(
                                    name=f"wsplit-{n}",
                                    engine=inst.engine,
                                    sync_info=mybir.SyncInfo(
                                        on_wait=[w], on_update=[]
                                    ),
                                    bass_nofuse=True,
                                )
                            )
                        inst.sync_info = mybir.SyncInfo(
                            on_wait=keep, on_update=si.on_update
                        )
                new.append(inst)
            blk.instructions[:] = new
    return n


def _prep_weights(kernel):
    # wt[32*rho + ci, 128*dx + 64*s + co] = kernel[co, ci, rho - s, dx]
    # for rho - s in {0,1,2}, else 0.
    w = np.asarray(kernel).astype(np.float16)  # [co, ci, dy, dx]
    wt = np.zeros((128, 384), dtype=np.float16)
    for rho in range(4):
        for s in range(2):
            dy = rho - s
            if 0 <= dy <= 2:
                for dx in range(3):
                    # [co, ci] -> [ci, co]
                    wt[
                        32 * rho : 32 * rho + 32,
                        128 * dx + 64 * s : 128 * dx + 64 * s + 64,
                    ] = w[:, :, dy, dx].T
    return wt


def _prep_input(input, bpc=BPC, h=H):
    # [N, CIN, h, W] f32 -> f16 [N//bpc, bpc, 2, CIN, (h//2+1)*WR]:
    # padded[ci, 1+r, 1+x] = in[ci, r, x] (zero border, rows -1..h,
    # cols -1..W); x[n, rho, ci, k*WR : (k+1)*WR] = padded[ci, 2k+rho, :]
    # i.e. image row 2k-1+rho as a 130-wide stored row.
    n = input.shape[0]
    nk = h // 2 + 1
    pad = np.zeros((n, CIN, h + 2, WR), dtype=np.float16)
    pad[:, :, 1 : h + 1, 1 : W + 1] = input
    xr = np.empty((n, 2, CIN, nk, WR), dtype=np.float16)
    for rho in range(2):
        xr[:, rho] = pad[:, :, rho : rho + 2 * nk : 2, :].transpose(0, 1, 2, 3)
    return np.ascontiguousarray(xr.reshape(n // bpc, bpc, 2, CIN, nk * WR))


def run(input, kernel, **spmd_kwargs):
    """Run the kernel on 8 NeuronCores; returns (output, BassKernelResults)."""
    from concourse.bass_utils import run_bass_kernel_spmd

    if "nc" not in _CACHE:
        _CACHE["nc"] = build_nc()
    nc = _CACHE["nc"]

    inp = _prep_input(np.asarray(input))
    wts = _prep_weights(np.asarray(kernel))
    in_maps = [{"x": inp[c], "w": wts} for c in range(NCORES)]
    bkr = run_bass_kernel_spmd(nc, in_maps, list(range(NCORES)), **spmd_kwargs)
    out = np.concatenate([bkr.results[c]["y"] for c in range(NCORES)], axis=0)
    return _unstage(out), bkr


def _unstage(y, h=H):
    # y [B, n_st, 128, 2048] f16 -> out [B, COUT, h, W] f32
    # y[b, g, 64s+co, 512q + 128tq + x] = out[b, co, 32g+8q+2tq+s, x]
    n = y.shape[0]
    a = y.reshape(n, h // 32, 2, 64, 4, 4, W)  # b, g, s, co, q, tq, x
    a = a.transpose(0, 3, 1, 4, 5, 2, 6)  # b, co, g, q, tq, s, x
    return np.ascontiguousarray(a.reshape(n, COUT, h, W), dtype=np.float32)


def kernel(input, kernel):
    return run(input, kernel)[0]


# revision 29
# speedup vs baseline: 1.2858x; 1.0386x over previous
"""Trainium2 Bass kernel for nn_CustomConv: 3x3 same-padding conv.

Full problem: input [32, 32, 128, 128] f32, weight [64, 32, 3, 3] f32
-> output [32, 64, 128, 128] f32.

Sharding: data-parallel across 8 NeuronCores on the batch axis (4 images
per core); the small weight tensor is replicated.

Per-core design ("rho row-pair" scheme; both DMA bytes and PE columns at
their structural minimum):
  * Contraction K = 128 = (rho, ci): rho in 0..3 indexes a 4-row input
    window, ci the 32 input channels. M = 128 = (s, co): each streamed
    rhs column produces BOTH output rows of a row pair (s in {0,1}) for
    all 64 output channels -> full 128-wide PE array from a single
    stream. The 3 dx taps are 3 PSUM-accumulating passes whose rhs is
    the same buffer offset by dx elements. Weights are block-banded
    (dy = rho - s in {0,1,2}), so 3 passes x 32768 pair-columns
    = 98k PE cycles/core, the same column count as a perfectly paired
    M=64 scheme but without relying on column-group concurrency.
  * Storage: partition (rho, ci) holds 130-wide stored rows
    [0 | row | 0] of image rows 2k-1+rho (every other row); the zero
    columns make all 3 dx passes read valid data (no wrap), and the
    zero pad rows live in DRAM. rho in {0,1} (65 rows/image) is loaded
    from HBM ONCE; rho in {2,3} is the SAME data shifted one stored
    row, generated on-chip by a single flat 64-partition row-aligned
    vector copy per unit (4x DVE perf-mode eligible; measured cheap).
    No memsets, no misaligned copies, no gpsimd (its big copies
    measured 10x slow and its activity stalls concurrent DVE).
  * HBM traffic: 4.3 MiB loads + 8.4 MiB f16 stores per core (vs
    21.5 MB for the 3-copy baseline, which was DMA-bound at ~21 GB/s
    per SDMA engine).
  * PSUM: one 4-bank [128,2048] tile per 32-row store group (bufs=2),
    evicted as two [128,1024] casts split DVE/ACT (halves the
    per-instruction eviction overhead vs four [128,512]).
  * ~4us of dependency-free junk matmuls at kernel start warm the PE
    HAM clock gate during the pipeline-fill loads.
"""

import numpy as np

import concourse.bass as bass
import concourse.mybir as mybir
from concourse.tile import TileContext

F32 = mybir.dt.float32
F16 = mybir.dt.float16

B, CIN, H, W = 32, 32, 128, 128
COUT, KS = 64, 3
NCORES = 8
BPC = B // NCORES  # images per core
WR = W + 2  # stored-row width: [0 | row | 0]
NK = H // 2 + 1  # stored rows per rho in {0,1} per image

_CACHE = {}


def build_nc(bpc=BPC, h=H, split_waits=True):
    """Build the per-core Bass module. bpc/h are parameterized only for
    small-scale simulation tests; hardware uses the defaults."""
    assert h % 64 == 0
    npair = h // 2
    n_st = h // 32  # store groups of 32 output rows
    nk = npair + 1  # stored rows per rho01 partition per image
    nc = bass.Bass()
    x = nc.declare_dram_parameter("x", [bpc, 2, CIN, nk * WR], F16, isOutput=False)
    wts = nc.declare_dram_parameter("w", [128, 384], F16, isOutput=False)
    # Staged output layout (host un-permutes):
    # y[b, g, 64*s + co, 512*q + 128*tq + x] = out[b, co, 32g+8q+2tq+s, x]
    y = nc.declare_dram_parameter("y", [bpc, n_st, 128, 2048], F16, isOutput=True)

    x_ap = x.ap().rearrange("b r c s -> b (r c) s")  # [bpc, 64, nk*WR]
    y_ap = y.ap()

    with TileContext(nc) as tc:
        with (
            tc.tile_pool(name="wpool", bufs=1) as wpool,
            tc.tile_pool(name="scpool", bufs=1) as scpool,
            tc.tile_pool(name="inpool", bufs=6) as inpool,
            tc.tile_pool(name="stpool", bufs=6) as stpool,
            tc.tile_pool(name="psum", bufs=4, space="PSUM") as psum_pool,
        ):
            wt = wpool.tile([128, 384], F16)
            nc.sync.dma_start(out=wt, in_=wts.ap())

            # HAM warmup: junk matmuls on an UNINITIALIZED scratch tile
            # (zero data deps, results discarded) run the moment the
            # engines come live, so the PE clock gate is already 8/8 when
            # the first real matmul's data lands (~3us later).
            scr = scpool.tile([128, 384], F16)
            nc.gpsimd.memset(scr[:, :], 0.0)
            wps = psum_pool.tile([128, 1024], F32, tag="ps", name="warm")
            for i in range(14):
                nc.tensor.matmul(
                    wps[:, 0:384],
                    lhsT=scr[:, 0:128],
                    rhs=scr,
                    start=True,
                    stop=True,
                    skip_group_check=True,
                )

            # Pipeline units (in units of output row-pairs): half-images,
            # with the first and last half-image split into quarters so
            # the first matmul starts after ~a quarter of the fill load
            # and the drain tail stores sooner.
            hp = npair // 2
            def units_for(b):
                if hp < 16:
                    return [(0, hp), (hp, hp)]
                first = [(0, 16), (16, 16)] if b == 0 else [(0, hp)]
                second = (
                    [(hp, 16), (hp + 16, 16)] if b == bpc - 1 else [(hp, hp)]
                )
                return first + second

            units = [
                (b, t0, np_) for b in range(bpc) for t0, np_ in units_for(b)
            ]

            def fetch(u, first=False):
                """Issue unit u's loads and rho23 shift-copies; return the
                buffer tile. Called one unit AHEAD of the matmul groups so
                the DVE copies aren't FIFO'd behind the previous unit's
                psum casts (measured 0.5-2.4us/unit of PE stall)."""
                b, t0, np_ = u
                lsz = (np_ + 1) * WR
                hnp = np_ // 2
                buf = inpool.tile([128, lsz], F16, tag="img", name="buf")
                # One load per unit (prefetch-ahead hides the latency;
                # fewer DMAs = fewer semaphores for the fixed epilogue
                # teardown). The FIRST unit splits in two row-aligned
                # chunks so its first half-copy (and the q-outer first
                # group) starts one chunk earlier.
                cuts = [0, (hnp + 1) * WR, lsz] if first else [0, lsz]
                for k in range(len(cuts) - 1):
                    nc.sync.dma_start(
                        out=buf[0:64, cuts[k] : cuts[k + 1]],
                        in_=x_ap[b][
                            :, t0 * WR + cuts[k] : t0 * WR + cuts[k + 1]
                        ],
                    )
                # rho2/rho3 = rho0/rho1 advanced one stored row: flat
                # 64-partition row-aligned copies (260B offset, 4x DVE
                # perf-mode eligible).
                nc.vector.tensor_copy(
                    out=buf[64:128, 0 : hnp * WR],
                    in_=buf[0:64, WR : (hnp + 1) * WR],
                )
                nc.vector.tensor_copy(
                    out=buf[64:128, hnp * WR : np_ * WR],
                    in_=buf[0:64, (hnp + 1) * WR : (np_ + 1) * WR],
                )
                return buf

            nbuf = fetch(units[0], first=True)
            for ui, (b, t0, np_) in enumerate(units):
                buf = nbuf
                if ui + 1 < len(units):
                    nbuf = fetch(units[ui + 1])
                bufv = buf.rearrange("p (k j) -> p k j", j=WR)
                if True:
                    for gl in range(np_ // 16):
                        g = t0 // 16 + gl
                        last = b == bpc - 1 and g == n_st - 1
                        st = stpool.tile([128, 2048], F16, tag="st")
                        # Two 2-bank psum tiles per group: q0/q1 banks are
                        # freed by the DVE cast ALONE and q2/q3 by the ACT
                        # activate ALONE, decoupling the WAR chain that a
                        # single 4-bank tile serializes on max(both).
                        psa = psum_pool.tile([128, 1024], F32, tag="ps", name="psa")
                        psb = psum_pool.tile([128, 1024], F32, tag="ps", name="psb")
                        # dx-outer keeps the static schedule clean (a
                        # q-outer early-evict variant measured WORSE: the
                        # list scheduler interleaves groups and delays the
                        # evictions the psum rotation waits on). Only the
                        # LAST group uses q-outer + split stores to
                        # shorten the drain tail (no competing groups
                        # left to mis-schedule).
                        first = ui == 0 and gl == 0
                        order = (
                            [(q, dx) for q in range(4) for dx in range(3)]
                            if last or first
                            else [(q, dx) for dx in range(3) for q in range(4)]
                        )
                        for q, dx in order:
                            kloc = 16 * gl + 4 * q
                            psq = psa if q < 2 else psb
                            nc.tensor.matmul(
                                psq[:, 512 * (q % 2) : 512 * (q % 2) + 512],
                                lhsT=wt[:, 128 * dx : 128 * dx + 128],
                                rhs=bufv[:, kloc : kloc + 4, dx : dx + W],
                                start=(dx == 0),
                                stop=(dx == 2),
                                skip_group_check=True,
                            )
                            if last and q == 1 and dx == 2:
                                nc.vector.tensor_copy(
                                    out=st[:, 0:1024], in_=psa
                                )
                                nc.sync.dma_start(
                                    out=y_ap[b, g][:, 0:1024],
                                    in_=st[:, 0:1024],
                                )
                        if last:
                            nc.scalar.copy(st[:, 1024:2048], psb)
                            nc.scalar.dma_start(
                                out=y_ap[b, g][:, 1024:2048],
                                in_=st[:, 1024:2048],
                            )
                        else:
                            nc.vector.tensor_copy(out=st[:, 0:1024], in_=psa)
                            nc.scalar.copy(st[:, 1024:2048], psb)
                            nc.scalar.dma_start(out=y_ap[b, g], in_=st)
    if split_waits:
        _split_waits(nc)
    return nc


# Per-instruction-struct HW sync-wait slot limits are small (walrus
# "Too many sync wait commands"). Split excess waits onto standalone
# NoOp instructions queued just before, on the same engine.
_WAIT_LIMIT = {}
_SKIP_SPLIT = {
    "InstEventSemaphore",
    "InstAllEngineBarrier",
    "InstUnconditionalBranch",
    "InstNoOp",
}


def _split_waits(nc):
    n = 0
    for f in nc.m.functions:
        for blk in f.blocks:
            new = []
            for inst in blk.instructions:
                si = getattr(inst, "sync_info", None)
                tname = type(inst).__name__
                if si is not None and si.on_wait and tname not in _SKIP_SPLIT:
                    limit = _WAIT_LIMIT.get(tname, 1)
                    if len(si.on_wait) > limit:
                        extra, keep = si.on_wait[:-limit], si.on_wait[-limit:]
                        for w in extra:
                            n += 1
                            new.append(
                                mybir.InstNoOp(
                                    name=f"wsplit-{n}",
                                    engine=inst.engine,
                                    sync_info=mybir.SyncInfo(
                                        on_wait=[w], on_update=[]
                                    ),
                                    bass_nofuse=True,
                                )
                            )
                        inst.sync_info = mybir.SyncInfo(
                            on_wait=keep, on_update=si.on_update
                        )
                new.append(inst)
            blk.instructions[:] = new
    return n


def _prep_weights(kernel):
    # wt[32*rho + ci, 128*dx + 64*s + co] = kernel[co, ci, rho - s, dx]
    # for rho - s in {0,1,2}, else 0.
    w = np.asarray(kernel).astype(np.float16)  # [co, ci, dy, dx]
    wt = np.zeros((128, 384), dtype=np.float16)
    for rho in range(4):
        for s in range(2):
            dy = rho - s
            if 0 <= dy <= 2:
                for dx in range(3):
                    # [co, ci] -> [ci, co]
                    wt[
                        32 * rho : 32 * rho + 32,
                        128 * dx + 64 * s : 128 * dx + 64 * s + 64,
                    ] = w[:, :, dy, dx].T
    return wt


def _prep_input(input, bpc=BPC, h=H):
    # [N, CIN, h, W] f32 -> f16 [N//bpc, bpc, 2, CIN, (h//2+1)*WR]:
    # padded[ci, 1+r, 1+x] = in[ci, r, x] (zero border, rows -1..h,
    # cols -1..W); x[n, rho, ci, k*WR : (k+1)*WR] = padded[ci, 2k+rho, :]
    # i.e. image row 2k-1+rho as a 130-wide stored row.
    n = input.shape[0]
    nk = h // 2 + 1
    pad = np.zeros((n, CIN, h + 2, WR), dtype=np.float16)
    pad[:, :, 1 : h + 1, 1 : W + 1] = input
    xr = np.empty((n, 2, CIN, nk, WR), dtype=np.float16)
    for rho in range(2):
        xr[:, rho] = pad[:, :, rho : rho + 2 * nk : 2, :].transpose(0, 1, 2, 3)
    return np.ascontiguousarray(xr.reshape(n // bpc, bpc, 2, CIN, nk * WR))


def run(input, kernel, **spmd_kwargs):
    """Run the kernel on 8 NeuronCores; returns (output, BassKernelResults)."""
    from concourse.bass_utils import run_bass_kernel_spmd

    if "nc" not in _CACHE:
        _CACHE["nc"] = build_nc()
    nc = _CACHE["nc"]

    inp = _prep_input(np.asarray(input))
    wts = _prep_weights(np.asarray(kernel))
    in_maps = [{"x": inp[c], "w": wts} for c in range(NCORES)]
    bkr = run_bass_kernel_spmd(nc, in_maps, list(range(NCORES)), **spmd_kwargs)
    out = np.concatenate([bkr.results[c]["y"] for c in range(NCORES)], axis=0)
    return _unstage(out), bkr


def _unstage(y, h=H):
    # y [B, n_st, 128, 2048] f16 -> out [B, COUT, h, W] f32
    # y[b, g, 64s+co, 512q + 128tq + x] = out[b, co, 32g+8q+2tq+s, x]
    n = y.shape[0]
    a = y.reshape(n, h // 32, 2, 64, 4, 4, W)  # b, g, s, co, q, tq, x
    a = a.transpose(0, 3, 1, 4, 5, 2, 6)  # b, co, g, q, tq, s, x
    return np.ascontiguousarray(a.reshape(n, COUT, h, W), dtype=np.float32)


def kernel(input, kernel):
    return run(input, kernel)[0]
